# revision 57
# baseline (speedup 1.0000x reference)
"""RadianceNetwork (256 independent channel MLPs) on 8 Trainium2 NeuronCores.

Expert-parallel: 32 channel MLPs per core. Each channel computes
    h1 = relu(x @ W1[c] + b1[c])      x: [64, 135], W1: [135, 256]
    h2 = relu(h1 @ W2[c] + b2[c])     W2: [256, 256]
    o  = h2 @ W3[c] + b3[c]           W3: [256, 816]
with x identical across channels. All three layers compute transposed
activations (hidden/output on partitions, batch on the free dim): each
layer's output feeds the next as matmul rhs with no transposes, every
matmul streams only B=64 rows (fp32 costs 4 PE cycles/row, so keeping
the moving dim small keeps the PE off the critical path), and every
bias is per-partition, folded into the PSUM->SBUF eviction on the
Vector engine. The kernel is memory-bound on the ~40 MB of per-core
weights: all per-channel weights (W2 | W3 | W1-main) are host-packed
into ONE 1.23 MB contiguous-per-partition DMA on the sync HWDGE queue
(one issue + one completion wait per channel keeps the SP sequencer off
the critical path), W1's 7 leftover K-rows for all 32 channels are a
one-time const, and output stores go through SWDGE on the otherwise
idle GpSimd queue. Channels are processed two at a time,
stage-interleaved, so the PE always has the sibling channel's matmuls
while ACT/DVE drain PSUM.
"""

import threading

import numpy as np

import concourse.bass as bass
import concourse.mybir as mybir
import concourse.tile as tile
from concourse.bass import MemorySpace
from concourse.bass_utils import run_bass_kernel_spmd

POS, VIEW, FEAT = 3, 3, 128
IN_DIM = POS + VIEW + FEAT + 1  # 135
HID = 256
N_SUB = 408
N_UE, N_BS = 4, 64
C = N_UE * N_BS  # 256
OUT_DIM = N_SUB * 2  # 816
B = 64
N_CORES = 8
NCH = C // N_CORES  # 32 channels per core
NJ = (OUT_DIM + 127) // 128  # 7 output chunks of <=128
TAIL = OUT_DIM - 128 * (NJ - 1)  # 48

F32 = mybir.dt.float32

_LOCK = threading.Lock()
_CACHE: dict = {}
LAST_RESULTS = None  # BassKernelResults of the most recent run (for test.py)


def _split_multi_waits(nc, max_waits=1):
    """The walrus build here rejects any instruction carrying more than
    one sync wait ("Too many sync wait commands"), while Tile routinely
    emits 2-4 waits per instruction. Hoist extra waits onto standalone
    single-wait instructions (same engine, immediately before), which is
    semantically identical: the engine's sequencer blocks on each in
    program order."""
    import bass_rust

    for fn in nc.m.functions:
        new_blocks = []
        changed = False
        for bb in fn.blocks:
            insts = list(bb.instructions)
            if not any(
                inst.sync_info is not None and len(inst.sync_info.on_wait) > max_waits
                for inst in insts
            ):
                new_blocks.append(bb)
                continue
            changed = True
            out = []
            for inst in insts:
                si = inst.sync_info
                if si is not None and len(si.on_wait) > max_waits:
                    waits = list(si.on_wait)
                    head, keep = waits[:-max_waits], waits[-max_waits:]
                    for j, w in enumerate(head):
                        out.append(
                            mybir.InstEventSemaphore(
                                name=f"{inst.name}-sw{j}",
                                engine=inst.engine,
                                ins=[],
                                outs=[],
                                sync_info=mybir.SyncInfo(on_wait=[w], on_update=[]),
                                bass_nofuse=True,
                            )
                        )
                    inst.sync_info = mybir.SyncInfo(
                        on_wait=keep, on_update=list(si.on_update)
                    )
                out.append(inst)
            new_blocks.append(
                bass_rust.BasicBlock(
                    name=bb.name,
                    instructions=out,
                    IsPredicated=bb.IsPredicated,
                    IsExit=bb.IsExit,
                    IsLoopEntry=bb.IsLoopEntry,
                )
            )
        if changed:
            fn.blocks = new_blocks


F_W2 = 0  # wp col offset of W2 [256]
F_W3 = HID  # wp col offset of W3 [816]
F_W1 = HID + OUT_DIM  # wp col offset of the W1 main-K half-chunk [128]
F_TOT = HID + OUT_DIM + 128  # 1200


def _build(w_bufs=6, out_bufs=4, h_bufs=3, dve_chunks=(0, 1, 2, 3, 4, 5, 6),
           ph_bufs=2, po_bufs=2, ilv=2, ph_pack=False, evict_bcast=True,
           store_mode="pool_main_sync_tail", w_split=False, preload=4,
           split_tail=0):
    nc = bass.Bass()
    xt = nc.declare_dram_parameter("xt", [IN_DIM, B], F32, isOutput=False)
    # wp[c, k, p, :] = [W2[c, k*128+p, :], W3[c, k*128+p, :], W1[c, p, k*128:(k+1)*128]]
    wp = nc.declare_dram_parameter("wp", [NCH, 2, 128, F_TOT], F32, isOutput=False)
    # w1r[r, c*256+h] = W1[c, 128+r, h] (the 7 leftover K rows of every channel)
    w1r = nc.declare_dram_parameter(
        "w1r", [IN_DIM - 128, NCH * HID], F32, isOutput=False
    )
    b1t = nc.declare_dram_parameter("b1t", [128, 2 * NCH], F32, isOutput=False)
    b2t = nc.declare_dram_parameter("b2t", [128, 2 * NCH], F32, isOutput=False)
    b3t = nc.declare_dram_parameter("b3t", [128, NJ * NCH], F32, isOutput=False)
    # out[c, p, j, b] = o_T[c][j*128+p, b] for j < 6; tail chunk separate
    if store_mode == "pool1":
        out = nc.declare_dram_parameter("out", [NCH, 128, NJ, B], F32, isOutput=True)
        outt = None
    else:
        out = nc.declare_dram_parameter(
            "out", [NCH, 128, NJ - 1, B], F32, isOutput=True
        )
        outt = nc.declare_dram_parameter("outt", [NCH, TAIL, B], F32, isOutput=True)

    with tile.TileContext(nc) as tc:
        with (
            tc.tile_pool(name="consts", bufs=1) as consts,
            tc.tile_pool(name="wpp", bufs=w_bufs) as wpp,
            tc.tile_pool(name="wsp", bufs=2) as wsp,
            tc.tile_pool(name="hp", bufs=h_bufs) as hp,
            tc.tile_pool(name="op", bufs=out_bufs) as op,
            tc.tile_pool(name="ph", bufs=ph_bufs, space=MemorySpace.PSUM) as php,
            tc.tile_pool(name="po", bufs=po_bufs, space=MemorySpace.PSUM) as pop,
        ):
            preloaded = {}
            if preload:
                # start the first weight streams before the (tiny) consts so
                # the DMA engines have bulk work from the first microsecond
                def _early_load(c):
                    wps = wpp.tile([128, 2, F_TOT], F32, tag="wp")
                    nc.sync.dma_start(
                        out=wps, in_=wp[c].rearrange("k p f -> p k f")
                    )
                    return wps

                def _early_load_split(c):
                    # first channels: two half-loads so the PE can start on
                    # the k=0 half ~1.4 us before the full stream lands
                    wk0 = wsp.tile([128, 1, F_TOT], F32, tag="wp0")
                    nc.sync.dma_start(out=wk0[:, 0, :], in_=wp[c, 0])
                    wk1 = wsp.tile([128, 1, F_TOT], F32, tag="wp1")
                    nc.sync.dma_start(out=wk1[:, 0, :], in_=wp[c, 1])

                    class _W:
                        def __getitem__(self, idx):
                            _, k, fs = idx
                            return (wk0 if k == 0 else wk1)[:, 0, fs]

                    return _W()

                for c in range(preload):
                    preloaded[c] = (
                        _early_load_split(c) if c < 2 else _early_load(c)
                    )
            xk_a = consts.tile([128, B], F32)
            nc.sync.dma_start(out=xk_a, in_=xt[0:128, :])
            xk_b = consts.tile([IN_DIM - 128, B], F32)
            nc.sync.dma_start(out=xk_b, in_=xt[128:IN_DIM, :])
            b1s = consts.tile([128, 2 * NCH], F32)
            nc.sync.dma_start(out=b1s, in_=b1t[:, :])
            b2s = consts.tile([128, 2 * NCH], F32)
            nc.sync.dma_start(out=b2s, in_=b2t[:, :])
            b3s = consts.tile([128, NJ * NCH], F32)
            nc.sync.dma_start(out=b3s, in_=b3t[:, :])
            w1rs = consts.tile([IN_DIM - 128, NCH * HID], F32)
            nc.sync.dma_start(out=w1rs, in_=w1r[:, :])

            relu = mybir.ActivationFunctionType.Relu

            def load_w(c):
                if not (w_split or c >= NCH - split_tail):
                    wps = wpp.tile([128, 2, F_TOT], F32, tag="wp")
                    nc.sync.dma_start(
                        out=wps, in_=wp[c].rearrange("k p f -> p k f")
                    )
                    return wps
                # tail channels (or w_split mode): two half-loads so the k=0
                # matmuls can start before the k=1 half lands; at the tail the
                # sync issue queue is empty, so the extra issue slot is free
                wk0 = wsp.tile([128, 1, F_TOT], F32, tag="wp0")
                nc.sync.dma_start(out=wk0[:, 0, :], in_=wp[c, 0])
                wk1 = wsp.tile([128, 1, F_TOT], F32, tag="wp1")
                nc.sync.dma_start(out=wk1[:, 0, :], in_=wp[c, 1])

                class _W:
                    def __getitem__(self, idx):
                        _, k, fs = idx
                        return (wk0 if k == 0 else wk1)[:, 0, fs]

                return _W()

            def layer1_mm(c, wps):
                # main K=128 rows from wp (m-chunk m lives in wp's k=m slot),
                # the 7 leftover K rows from the preloaded w1r const
                if ph_pack:
                    phx = php.tile([128, 4, B], F32, tag="phx")
                    ph1 = phx[:, 0:2, :]
                else:
                    phx = None
                    ph1 = php.tile([128, 2, B], F32, tag="ph1")
                for m in range(2):
                    nc.tensor.matmul(
                        ph1[:, m, :], wps[:, m, F_W1 : F_W1 + 128], xk_a,
                        start=True, stop=False,
                    )
                    nc.tensor.matmul(
                        ph1[:, m, :],
                        w1rs[:, HID * c + m * 128 : HID * c + (m + 1) * 128],
                        xk_b, start=False, stop=True,
                    )
                return ph1, phx

            def _relu_pair(c, src, h, bs):
                # both chunks on ACT: offloading one to DVE was tested and is
                # slightly worse (DVE's in-order queue delays the evictions)
                for m in range(2):
                    nc.scalar.activation(
                        h[:, m, :], src[:, m, :], relu,
                        bias=bs[:, 2 * c + m : 2 * c + m + 1],
                    )

            def layer1_act(c, ph1):
                h1 = hp.tile([128, 2, B], F32, tag="h1")
                _relu_pair(c, ph1, h1, b1s)
                return h1

            def layer2_mm(c, wps, h1, phx=None):
                if ph_pack:
                    ph2 = phx[:, 2:4, :]
                else:
                    ph2 = php.tile([128, 2, B], F32, tag="ph2")
                for m in range(2):
                    ms = slice(F_W2 + m * 128, F_W2 + (m + 1) * 128)
                    for k in range(2):
                        nc.tensor.matmul(
                            ph2[:, m, :], wps[:, k, ms], h1[:, k, :],
                            start=(k == 0), stop=(k == 1),
                        )
                return ph2

            def layer2_act(c, ph2):
                h2 = hp.tile([128, 2, B], F32, tag="h2")
                _relu_pair(c, ph2, h2, b2s)
                return h2

            def layer3_mm(c, wps, h2):
                po = pop.tile([128, NJ, B], F32, tag="po")
                for j in range(NJ):
                    pj = 128 if j < NJ - 1 else TAIL
                    js = slice(F_W3 + j * 128, F_W3 + j * 128 + pj)
                    for k in range(2):
                        nc.tensor.matmul(
                            po[:pj, j, :], wps[:, k, js], h2[:, k, :],
                            start=(k == 0), stop=(k == 1),
                        )
                return po

            def layer3_evict(c, po):
                # bias-add rides the PSUM->SBUF eviction (Scalar or Vector);
                # osb is j-major so each evict writes a contiguous 256 B run
                # and the store DMA reads one contiguous 1536 B/partition
                # block (+ a small separate tail-chunk store)
                osb = op.tile([128, NJ, B], F32, tag="osb")
                # (split evict+store halves for the last channels was tested:
                # +330 ns — the extra issue slots outweigh the overlap)
                if evict_bcast:
                    # one DVE op: osb[p, j, b] = po[p, j, b] + b3[p, j]
                    # (bias broadcast along b with a stride-0 AP; the unwritten
                    # po tail rows produce garbage that the stores never read)
                    bias_b = b3s[:, NJ * c : NJ * (c + 1), None].broadcast_to(
                        [128, NJ, B]
                    )
                    nc.vector.tensor_add(osb, po, bias_b)
                else:
                    for j in range(NJ):
                        pj = 128 if j < NJ - 1 else TAIL
                        bcol = b3s[:pj, NJ * c + j : NJ * c + j + 1]
                        if (j in dve_chunks) if dve_chunks is not None else (c % 2 == 0):
                            nc.vector.tensor_scalar_add(
                                osb[:pj, j, :], po[:pj, j, :], bcol
                            )
                        else:
                            nc.scalar.add(osb[:pj, j, :], po[:pj, j, :], bcol)
                # the final channels' stores go fully on sync: its queue is
                # idle once all weight loads are issued, and this skips the
                # ~1 us SWDGE descriptor-gen on the drain critical path
                tail_sync = c >= NCH - 4
                if store_mode == "pool1":
                    nc.gpsimd.dma_start(out=out[c], in_=osb)
                elif store_mode == "pool2":
                    nc.gpsimd.dma_start(out=out[c], in_=osb[:, 0 : NJ - 1, :])
                    nc.gpsimd.dma_start(out=outt[c], in_=osb[:TAIL, NJ - 1, :])
                elif store_mode == "pool_main_sync_tail":
                    eng = nc.sync if tail_sync else nc.gpsimd
                    eng.dma_start(out=out[c], in_=osb[:, 0 : NJ - 1, :])
                    nc.sync.dma_start(out=outt[c], in_=osb[:TAIL, NJ - 1, :])
                else:  # sync2
                    nc.sync.dma_start(out=out[c], in_=osb[:, 0 : NJ - 1, :])
                    nc.sync.dma_start(out=outt[c], in_=osb[:TAIL, NJ - 1, :])

            # ilv channels in flight, stage-interleaved so the PE always has
            # sibling channels' matmuls while ACT/DVE drain PSUM
            for c0 in range(0, NCH, ilv):
                grp = list(range(c0, min(c0 + ilv, NCH)))
                ws = [
                    preloaded.pop(c) if c in preloaded else load_w(c)
                    for c in grp
                ]
                p1 = [layer1_mm(c, w) for c, w in zip(grp, ws)]
                h1s = [layer1_act(c, p) for c, (p, _) in zip(grp, p1)]
                p2 = [
                    layer2_mm(c, w, h, px)
                    for c, w, h, (_, px) in zip(grp, ws, h1s, p1)
                ]
                h2s = [layer2_act(c, p) for c, p in zip(grp, p2)]
                p3 = [layer3_mm(c, w, h) for c, w, h in zip(grp, ws, h2s)]
                for c, p in zip(grp, p3):
                    layer3_evict(c, p)

    _split_multi_waits(nc)
    return nc


def _prep_inputs(ue_positions, view_directions, spatial_features, bs_antenna_ids,
                 W1, b1, W2, b2, W3, b3):
    ue_positions = np.asarray(ue_positions, np.float32)
    view_directions = np.asarray(view_directions, np.float32)
    spatial_features = np.asarray(spatial_features, np.float32)
    ids = np.asarray(bs_antenna_ids)
    nid = (ids.astype(np.float32) - 1.0) / (N_BS - 1)
    x = np.concatenate(
        [ue_positions, view_directions, spatial_features, nid[:, None]], axis=1
    )
    xt = np.ascontiguousarray(x.T)  # [135, 64]

    W1 = np.asarray(W1, np.float32)
    W2 = np.asarray(W2, np.float32)
    W3 = np.asarray(W3, np.float32)
    b1 = np.asarray(b1, np.float32)
    b2 = np.asarray(b2, np.float32)
    b3 = np.asarray(b3, np.float32)

    in_maps = []
    for i in range(N_CORES):
        cs = slice(i * NCH, (i + 1) * NCH)
        # wp[c, k, p, :] = [W2[c, k*128+p, :], W3[c, k*128+p, :],
        #                   W1[c, p, k*128:(k+1)*128]]
        wpk = np.concatenate(
            [
                W2[cs].reshape(NCH, 2, 128, HID),
                W3[cs].reshape(NCH, 2, 128, OUT_DIM),
                np.ascontiguousarray(
                    W1[cs, :128, :].reshape(NCH, 128, 2, 128).transpose(0, 2, 1, 3)
                ),
            ],
            axis=3,
        )
        # w1r[r, c*256+h] = W1[c, 128+r, h]
        w1r = np.ascontiguousarray(
            W1[cs, 128:, :].transpose(1, 0, 2).reshape(IN_DIM - 128, NCH * HID)
        )
        # b{1,2}t[p, c*2+m] = b[c, m*128+p]
        b1t = np.ascontiguousarray(
            b1[cs].reshape(NCH, 2, 128).transpose(2, 0, 1).reshape(128, 2 * NCH)
        )
        b2t = np.ascontiguousarray(
            b2[cs].reshape(NCH, 2, 128).transpose(2, 0, 1).reshape(128, 2 * NCH)
        )
        # b3t[p, c*NJ+j] = b3[c, j*128+p] (zero-padded tail)
        b3p = np.zeros((NCH, NJ, 128), np.float32)
        b3p.reshape(NCH, NJ * 128)[:, :OUT_DIM] = b3[cs]
        b3t = np.ascontiguousarray(
            b3p.transpose(2, 0, 1).reshape(128, NJ * NCH)
        )
        in_maps.append(
            dict(
                xt=xt,
                wp=np.ascontiguousarray(wpk),
                w1r=w1r,
                b1t=b1t,
                b2t=b2t,
                b3t=b3t,
            )
        )
    return in_maps


def kernel(ue_positions, view_directions, spatial_features, bs_antenna_ids,
           W1, b1, W2, b2, W3, b3):
    global LAST_RESULTS
    with _LOCK:
        if "nc" not in _CACHE:
            _CACHE["nc"] = _build()
    nc = _CACHE["nc"]
    in_maps = _prep_inputs(
        ue_positions, view_directions, spatial_features, bs_antenna_ids,
        W1, b1, W2, b2, W3, b3,
    )
    try:
        res = run_bass_kernel_spmd(nc, in_maps, list(range(N_CORES)))
    except ModuleNotFoundError:
        # BASS_TRACE=1 + axon NTFF hook absent (antenv.axon_hooks) raises
        # before execution; rerun with tracing hard-disabled.
        import os

        os.environ["BASS_NEVER_TRACE"] = "1"
        res = run_bass_kernel_spmd(nc, in_maps, list(range(N_CORES)))
    LAST_RESULTS = res
    # out[c, p, j, b] -> oT[c, j*128+p, b]; outt (if present) holds rows 768:816
    parts = []
    for i in range(N_CORES):
        arr = res.results[i]["out"]  # [NCH, 128, NJ-1 or NJ, B]
        nj = arr.shape[2]
        main = arr.transpose(0, 2, 1, 3).reshape(NCH, nj * 128, B)
        if "outt" in res.results[i]:
            parts.append(np.concatenate([main, res.results[i]["outt"]], axis=1))
        else:
            parts.append(main[:, :OUT_DIM, :])
    full = np.concatenate(parts, axis=0)  # [C, OUT_DIM, B]
    o = np.ascontiguousarray(full.transpose(2, 0, 1)).reshape(
        B, N_UE, N_BS, N_SUB, 2
    )
    return o.view(np.complex64)[..., 0]
